# revision 39
# baseline (speedup 1.0000x reference)
"""Trainium2 Bass kernel for nn_CrossAttention_84310208020733.

Cross-attention: out = proj(softmax(mask(q @ k^T * scale)) @ v), with
  q = tgt @ q_w.T + q_b               [B=4, NT=1024, D=1024]
  k, v = split(src @ kv_w.T + kv_b)   [B=4, NS=2048, D=1024], H=16 heads, Dh=64

Sharding over 8 NeuronCores: core c handles batch b = c//2 and head group
g = c%2 (8 heads = 512 channels).  Each core computes its partial
proj-output (contraction over its 512 attn channels) in transposed layout
[out_ch, rows]; the host sums the two partials per batch, transposes, and
adds proj_b (the "all-reduce after proj" done at gather time).

On-device layout is feature-major throughout ("T" = channels on SBUF
partitions):
  qT = qwT.T @ tgtT       [512, 1024]
  kT = kwT.T @ srcT       [512, NS_kept]
  v  = srcT.T @ vwT       [NS_kept, 512]   (+ ones/zero columns for row-sums)
  sT = kT_h.T @ qT_h      [src 128, rows 512] per head pair (row-packed K=64)
  pT = exp(sT * scale + maskbias)  (ACT, bf16 out; no max-subtraction)
  av = [v_h | 1].T @ pT   -> [Dh(+1), rows] unnormalized out + row sums
  oT = av * bcast(1/sum)  [512, 1024]
  outT = pwT.T @ oT       [1024, 1024] partial, bf16

Perf notes (measured on HW):
  - serial K=128 matmul chains run at ~318 ns per 512-col stream; two
    interleaved independent chains run at ~263 ns; K=64 quad pairs
    (tile_position (0,0)/(64,0)) co-stream at ~127 ns per matmul.  All
    projection chains are therefore emitted as interleaved pairs.
  - DMAs are issued in critical-path order (small consts, then per-k
    w_q/tgt-n0 pairs, w_k/src-col0 pairs, ...) so the first attention
    block starts ~15 us in instead of ~47 us.
  - GPSIMD cannot touch PSUM; fp8 fails the 2e-2 accuracy gate; both
    were measured/verified and rejected.

Fully-masked 128-wide src chunks (per the runtime mask, intersected across
batches) are dropped at compile time; partial masks are handled via the
additive -30000 bias inside the exp activation.
"""

import numpy as np
import ml_dtypes

import concourse.bass as bass
import concourse.bacc as bacc
import concourse.tile as tile
from concourse import mybir
from concourse.bass_utils import run_bass_kernel_spmd

P = 128
B = 4
NT = 1024
NS = 2048
D = 1024
H = 16
DH = 64
G = 2              # head groups (tensor-parallel dim)
HG = H // G        # heads per core = 8
CH = HG * DH       # channels per core = 512
KO = D // P        # 8 contraction chunks for the projections
CHO = CH // P      # 4 channel tiles per core
SCALE = DH ** -0.5
NEG = -30000.0
BF16 = mybir.dt.bfloat16
F32 = mybir.dt.float32
EXP = mybir.ActivationFunctionType.Exp

# vaug per-pair block: [A: 64 ch + 1 ones][B: 1 ones + 63 zero + 64 ch]
ABLK = DH + 1            # 65
BBLK = P                 # 128
PBLK = ABLK + BBLK       # 193


def _build_nc(nk: int) -> "bacc.Bacc":
    """Emit the per-core program for nk kept 128-wide source chunks."""
    ns_k = nk * P
    NSB = ns_k // 512 if ns_k % 512 == 0 else ns_k // 512 + 1
    nc = bacc.Bacc("TRN2", target_bir_lowering=False, debug=False)

    tgtT = nc.dram_tensor("tgtT", [D, NT], BF16, kind="ExternalInput")
    srcT = nc.dram_tensor("srcT", [D, ns_k], BF16, kind="ExternalInput")
    qwT = nc.dram_tensor("qwT", [D, CH], BF16, kind="ExternalInput")
    kwT = nc.dram_tensor("kwT", [D, CH], BF16, kind="ExternalInput")
    vwT = nc.dram_tensor("vwT", [D, CH], BF16, kind="ExternalInput")
    pwT = nc.dram_tensor("pwT", [CH, D], BF16, kind="ExternalInput")
    qb = nc.dram_tensor("qb", [CH], F32, kind="ExternalInput")
    kb = nc.dram_tensor("kb", [CH], F32, kind="ExternalInput")
    vb = nc.dram_tensor("vb", [CH], F32, kind="ExternalInput")
    maskT = nc.dram_tensor("maskT", [P, nk], F32, kind="ExternalInput")
    outT = nc.dram_tensor("outT", [D, NT], BF16, kind="ExternalOutput")

    with tile.TileContext(nc) as tc:
        with (
            tc.tile_pool(name="persist", bufs=1) as pers,
            tc.tile_pool(name="work", bufs=3) as work,
            tc.tile_pool(name="ps", bufs=2, space="PSUM") as ps,
        ):
            # ---- persistent tiles ------------------------------------
            # DMA issuing is the input-bandwidth ceiling (~616 ns per
            # dma_start, ~12.8 GB/s per DMA engine): keep per-k splits so
            # transfers spread over many engines, and round-robin the issue
            # cost over four otherwise-idle sequencers in priority order.
            mask_t = pers.tile([P, nk], F32, tag="mask_t")
            nc.scalar.dma_start(out=mask_t[:], in_=maskT.ap())
            qb_t = pers.tile([P, CHO], F32, tag="qb_t")
            nc.scalar.dma_start(out=qb_t[:], in_=qb.ap().rearrange("(o p) -> p o", p=P))
            kb_t = pers.tile([P, CHO], F32, tag="kb_t")
            nc.scalar.dma_start(out=kb_t[:], in_=kb.ap().rearrange("(o p) -> p o", p=P))
            vb_bc = pers.tile([P, CH], F32, tag="vb_bc")
            vb_ap = vb.ap()
            vb_bcast_src = bass.AP(tensor=vb_ap.tensor, offset=vb_ap.offset,
                                   ap=[[0, P]] + list(vb_ap.ap))
            nc.scalar.dma_start(out=vb_bc[:], in_=vb_bcast_src)

            w_q = pers.tile([P, KO, CH], BF16, tag="w_q")
            tgt_t = pers.tile([P, KO, NT], BF16, tag="tgt_t")
            w_k = pers.tile([P, KO, CH], BF16, tag="w_k")
            src_t = pers.tile([P, KO, ns_k], BF16, tag="src_t")
            w_v = pers.tile([P, KO, CH], BF16, tag="w_v")
            w_p = pers.tile([P, CHO, D], BF16, tag="w_p")
            qT = pers.tile([P, CHO, NT], BF16, tag="qT")
            kT = pers.tile([P, CHO, ns_k], BF16, tag="kT")
            oT = pers.tile([P, CHO, NT], BF16, tag="oT")

            # Each issuing engine's queues process its own backlog roughly
            # concurrently, so priority only holds WITHIN one engine's issue
            # order.  Assign whole waves per engine: sync owns the q-side
            # critical path, gpsimd owns the k/v-side src stream.  (Only
            # SP/gpsimd/ACT may issue DMAs; ACT is kept free for the exps.)
            # Each issuing engine feeds ~8 DMA queues (~100 GB/s), and an
            # engine's queued DMAs transfer roughly in issue order.  So: split
            # every wave half/half between sync and gpsimd (both run at full
            # tilt, waves complete in priority order at ~200 GB/s aggregate).
            def dma_wave(out_slices):
                for idx, (out, in_) in enumerate(out_slices):
                    eng = nc.sync if idx % 2 == 0 else nc.gpsimd
                    eng.dma_start(out=out, in_=in_)

            # k=0 slices split 4-ways: a single 128 KB DMA sits on one ~13 GB/s
            # queue for ~10 us, which would gate the very first matmuls
            c0w = min(512, ns_k)
            dma_wave([(w_q[:, 0, c * 128:(c + 1) * 128],
                       qwT.ap()[0:P, c * 128:(c + 1) * 128]) for c in range(4)]
                     + [(tgt_t[:, 0, c * 128:(c + 1) * 128],
                         tgtT.ap()[0:P, c * 128:(c + 1) * 128]) for c in range(4)])
            dma_wave([(w_q[:, k, :], qwT.ap()[k * P:(k + 1) * P, :])
                      for k in range(1, KO)]
                     + [(tgt_t[:, k, 0:512], tgtT.ap()[k * P:(k + 1) * P, 0:512])
                        for k in range(1, KO)])
            dma_wave([(w_k[:, 0, c * 128:(c + 1) * 128],
                       kwT.ap()[0:P, c * 128:(c + 1) * 128]) for c in range(4)]
                     + [(src_t[:, 0, c * 128:(c + 1) * 128],
                         srcT.ap()[0:P, c * 128:(c + 1) * 128])
                        for c in range(min(4, (c0w + 127) // 128))])
            dma_wave([(w_k[:, k, :], kwT.ap()[k * P:(k + 1) * P, :])
                      for k in range(1, KO)]
                     + [(src_t[:, k, 0:c0w],
                         srcT.ap()[k * P:(k + 1) * P, 0:c0w])
                        for k in range(1, KO)])
            dma_wave([(w_v[:, k, :], vwT.ap()[k * P:(k + 1) * P, :])
                      for k in range(KO)])
            for c0 in range(512, ns_k, 512):
                c1 = min(c0 + 512, ns_k)
                dma_wave([(src_t[:, k, c0:c1],
                           srcT.ap()[k * P:(k + 1) * P, c0:c1])
                          for k in range(KO)])
            dma_wave([(tgt_t[:, k, 512:1024],
                       tgtT.ap()[k * P:(k + 1) * P, 512:1024])
                      for k in range(KO)])
            dma_wave([(w_p[:, o, :], pwT.ap()[o * P:(o + 1) * P, :])
                      for o in range(CHO)])

            vaug = [pers.tile([P, HG // 2 * PBLK], BF16, tag=f"vaug{i}",
                              name=f"vaug{i}")
                    for i in range(nk)]

            # ---- emission units --------------------------------------
            # Interleaved pair-chains: emit two independent K-accumulation
            # chains with their matmuls alternated so the PE pipelines the
            # weight loads (measured ~263 ns vs ~318 ns per matmul).
            def _mm_pair(specs):
                # specs: list of (pmm, lhsT_fn, rhs_fn, n_k) per chain
                depth = max(s[3] for s in specs)
                for k in range(depth):
                    for pmm, lf, rf, nkk in specs:
                        if k < nkk:
                            nc.tensor.matmul(pmm[:], lf(k), rf(k),
                                             start=(k == 0), stop=(k == nkk - 1))

            def qt_chain(m, n):
                pmm = ps.tile([P, 512], F32, tag="acc", bufs=2, name="pmm_q")
                return (pmm,
                        lambda k: w_q[:, k, m * P:(m + 1) * P],
                        lambda k: tgt_t[:, k, n * 512:(n + 1) * 512], KO,
                        lambda: nc.vector.tensor_scalar_add(
                            qT[:, m, n * 512:(n + 1) * 512], pmm[:],
                            qb_t[:, m:m + 1]))

            def kt_chain(m, n):
                c0, c1 = n * 512, min((n + 1) * 512, ns_k)
                pmm = ps.tile([P, 512], F32, tag="acc", bufs=2, name="pmm_k")
                pv = pmm[:, 0:c1 - c0]
                return (pv,
                        lambda k: w_k[:, k, m * P:(m + 1) * P],
                        lambda k: src_t[:, k, c0:c1], KO,
                        lambda: nc.vector.tensor_scalar_add(
                            kT[:, m, c0:c1], pv[:], kb_t[:, m:m + 1]))

            def v_chain(ms):
                pmm = ps.tile([P, 512], F32, tag="acc", bufs=2, name="pmm_v")

                def fin():
                    va = vaug[ms].rearrange("p (t c) -> p t c", c=PBLK)
                    pv = pmm.rearrange("p (t c) -> p t c", c=2 * DH)
                    vv = vb_bc.rearrange("p (t c) -> p t c", c=2 * DH)
                    nc.vector.tensor_add(va[:, :, 0:DH], pv[:, :, 0:DH],
                                         vv[:, :, 0:DH])
                    nc.vector.tensor_add(va[:, :, ABLK + DH:PBLK],
                                         pv[:, :, DH:2 * DH],
                                         vv[:, :, DH:2 * DH])
                    nc.vector.memset(va[:, :, DH:DH + 1], 1.0)
                    nc.vector.memset(va[:, :, ABLK:ABLK + 1], 1.0)
                    nc.vector.memset(va[:, :, ABLK + 1:ABLK + DH], 0.0)
                return (pmm,
                        lambda k: src_t[:, k, ms * P:(ms + 1) * P],
                        lambda k: w_v[:, k, :], KO, fin)

            def proj_chain(m, n, tag="acc", act_copy=False):
                if tag == "st":
                    pmm = ps.tile([P, 1024], F32, tag="st", name="pmm_p")[:, 0:512]
                else:
                    pmm = ps.tile([P, 512], F32, tag=tag, bufs=2, name="pmm_p")

                def fin():
                    ob = work.tile([P, 512], BF16, tag="ob", bufs=4)
                    if act_copy:
                        nc.scalar.copy(ob[:], pmm[:])
                    else:
                        nc.vector.tensor_copy(ob[:], pmm[:])
                    # 4 column sub-DMAs: one 128 KB DMA would sit on a single
                    # ~13 GB/s queue for ~10 us.  n=0 stays on sync (gpsimd
                    # runs the norm broadcasts mid-sweep); the tail rotates
                    # over all three issuers.
                    for c in range(4):
                        eng = ([nc.sync, nc.gpsimd, nc.scalar][(m * 4 + c) % 3]
                               if n == 1 else nc.sync)
                        eng.dma_start(
                            out=outT.ap()[m * P:(m + 1) * P,
                                          n * 512 + c * 128:n * 512 + (c + 1) * 128],
                            in_=ob[:, c * 128:(c + 1) * 128])
                return (pmm,
                        lambda k: w_p[:, k, m * P:(m + 1) * P],
                        lambda k: oT[:, k, n * 512:(n + 1) * 512], CHO, fin)

            def pair_unit(*chains):
                def run():
                    ss = [c() for c in chains if c is not None]
                    _mm_pair([s[0:4] for s in ss])
                    for s in ss:
                        s[4]()
                    return True
                return run

            # filler queues: PE work woven between the attention chunks.
            crit = []
            lazy = []

            def drain(nu):
                # pop until a unit emits real work (qt1 units may be no-ops)
                for _ in range(nu):
                    while True:
                        q = crit if crit else lazy
                        if not q:
                            return
                        if q.pop(0)():
                            break

            def attn_block(t, n, v_weave=False, drain_js=(2, 6, 10),
                           pre_norm=None):
                # Emission in chunk pairs: both chunks' score quads go out
                # back-to-back (K=64 quads co-stream across adjacent matmuls),
                # then the av matmuls of the pair from two chunks ago (the lag
                # also absorbs late vaug/w_v arrival in block 0).
                rsl = slice(n * 512, (n + 1) * 512)
                avA = ps.tile([ABLK, 512], F32, tag="av", bufs=2, name="avA")
                avB = ps.tile([P, 512], F32, tag="av", bufs=2, name="avB")
                pts = {}

                def scores_exp(j):
                    st = ps.tile([P, 1024], F32, tag="st", name="st")
                    nc.tensor.matmul(
                        st[:, 0:512], kT[0:DH, t, j * P:(j + 1) * P],
                        qT[0:DH, t, rsl], start=True, stop=True,
                        tile_position=(0, 0))
                    nc.tensor.matmul(
                        st[:, 512:1024], kT[DH:P, t, j * P:(j + 1) * P],
                        qT[DH:P, t, rsl], start=True, stop=True,
                        tile_position=(64, 0))
                    pt = work.tile([P, 1024], BF16, tag="pt", bufs=6, name="pt")
                    nc.scalar.activation(out=pt[:], in_=st[:], func=EXP,
                                         bias=mask_t[:, j:j + 1], scale=SCALE)
                    pts[j] = pt

                def av(j):
                    pt = pts.pop(j)
                    va = vaug[j].rearrange("p (t c) -> p t c", c=PBLK)
                    nc.tensor.matmul(avA[:], va[:, t, 0:ABLK], pt[:, 0:512],
                                     start=(j == 0), stop=(j == nk - 1))
                    nc.tensor.matmul(avB[:], va[:, t, ABLK:PBLK],
                                     pt[:, 512:1024],
                                     start=(j == 0), stop=(j == nk - 1))

                # av lags 2 chunks (absorbs late vaug in block 0 and the
                # previous block's norm holding the av PSUM ring); the filler
                # drain sits before the avs so the PE has work if av waits
                for j0 in range(0, nk, 2):
                    j1 = min(j0 + 1, nk - 1)
                    scores_exp(j0)
                    if j1 > j0:
                        scores_exp(j1)
                    if v_weave:
                        pair_unit(mk(v_chain, j0),
                                  mk(v_chain, j1) if j1 > j0 else None)()
                    if j0 in drain_js:
                        drain(1)
                    for j in (j0 - 2, j0 - 1):
                        if j >= 0:
                            av(j)
                for j in sorted(pts):
                    av(j)
                if pre_norm is not None:
                    pre_norm()
                # normalization — no PE involvement: reciprocal of the [1,512]
                # sums row, Pool partition-broadcast, then a PSUM-direct mult.
                # Keeping the PE queue free here removes the inter-block bubble
                # where the next block's scores sat behind broadcast matmuls.
                halves = ((avA, avA[DH:DH + 1, :], avA[0:DH, :], 0, DH),
                          (avB, avB[0:1, :], avB[DH:P, :], DH, P))
                sums, rbrow, rb = [], [], []
                for _, srow, _, _, _ in halves:
                    su = work.tile([1, 512], F32, tag="sums", bufs=4,
                                   name="sums")
                    nc.vector.tensor_copy(su[:], srow)
                    sums.append(su)
                for su in sums:
                    rr = work.tile([1, 512], F32, tag="rbrow", bufs=4,
                                   name="rbrow")
                    nc.vector.reciprocal_approx_fast(rr[:], su[:])
                    rbrow.append(rr)
                for rr in rbrow:
                    rbt = work.tile([P, 512], F32, tag="rb", bufs=4, name="rb")
                    nc.gpsimd.partition_broadcast(rbt[:], rr[:])
                    rb.append(rbt)
                for (acc, _, data, r0, r1), rbt in zip(halves, rb):
                    nc.vector.tensor_mul(oT[r0:r1, t, rsl], data, rbt[r0:r1, :])

            # ---- schedule --------------------------------------------
            def mk(f, *a):
                return lambda: f(*a)

            qt1_done = [False] * CHO

            def qt1_pair(t):
                # covers qt(t,1) and qt(t+1,1) when available
                todo = [tt for tt in (t, t + 1) if tt < CHO and not qt1_done[tt]]
                for tt in todo:
                    qt1_done[tt] = True
                if todo:
                    pair_unit(*[mk(qt_chain, tt, 1) for tt in todo])()
                return bool(todo)

            # head: the window is DMA-paced.  qt(0,0) first (its inputs are
            # wave 1), then the other qt(t,0) chains fill the PE while wave 2
            # (kt inputs) streams in; kt(0,0) follows; kt(0,1)/kt(0,2) go to
            # the crit queue, drained inside block 0 before chunk 4 needs them.
            pair_unit(mk(qt_chain, 0, 0))()
            for t2 in range(1, CHO, 2):
                pair_unit(mk(qt_chain, t2, 0),
                          mk(qt_chain, t2 + 1, 0) if t2 + 1 < CHO else None)()
            pair_unit(mk(kt_chain, 0, 0))()
            for x in range(1, NSB, 2):
                crit.append(pair_unit(
                    mk(kt_chain, 0, x),
                    mk(kt_chain, 0, x + 1) if x + 1 < NSB else None))

            # n=0 sweep
            for t in range(CHO):
                if t + 1 < CHO:
                    for x in range(0, NSB, 2):
                        crit.append(pair_unit(
                            mk(kt_chain, t + 1, x),
                            mk(kt_chain, t + 1, x + 1) if x + 1 < NSB else None))
                lazy.append(mk(qt1_pair, t))
                attn_block(t, 0, v_weave=(t == 0))
                while crit:
                    crit.pop(0)()

            # n=1 sweep with n=0 projection woven in (one real unit per
            # ACT-bound block; the smart drain skips spent qt1 entries)
            for m in range(0, KO, 2):
                lazy.append(pair_unit(mk(proj_chain, m, 0),
                                      mk(proj_chain, m + 1, 0)))
            for t in range(CHO):
                qt1_pair(t)
                if t == CHO - 1:
                    # overlap the last block's normalization with four tail
                    # chains' partial contraction (k < CHO-1 only touches oT
                    # written by earlier blocks; m2/m3 borrow dead st slots)
                    head_specs = []

                    def tail_head():
                        for m, tg in ((0, "acc"), (1, "acc"),
                                      (2, "st"), (3, "st")):
                            head_specs.append(proj_chain(m, 1, tg, False))
                        for k in range(CHO - 1):
                            for s in head_specs:
                                nc.tensor.matmul(s[0][:], s[1](k), s[2](k),
                                                 start=(k == 0), stop=False)
                    attn_block(t, 1, drain_js=(6,), pre_norm=tail_head)
                else:
                    attn_block(t, 1, drain_js=(6,))
            while lazy:
                lazy.pop(0)()
            for s in head_specs:
                nc.tensor.matmul(s[0][:], s[1](CHO - 1), s[2](CHO - 1),
                                 start=False, stop=True)
            for i, s in enumerate(head_specs):
                s[4]()
            pair_unit(mk(proj_chain, 4, 1, "acc", True),
                      mk(proj_chain, 5, 1, "av", False),
                      mk(proj_chain, 6, 1, "av", True),
                      mk(proj_chain, 7, 1, "acc", False))()
    nc.compile()
    return nc


_NC_CACHE: dict[int, "bacc.Bacc"] = {}


def kernel(tgt, src, src_padded_mask, q_w, q_b, kv_w, kv_b, proj_w, proj_b,
           _run_kwargs: dict | None = None):
    tgt = np.asarray(tgt, dtype=np.float32)
    src = np.asarray(src, dtype=np.float32)
    mask = np.asarray(src_padded_mask).astype(bool)
    q_w = np.asarray(q_w, dtype=np.float32)
    q_b = np.asarray(q_b, dtype=np.float32)
    kv_w = np.asarray(kv_w, dtype=np.float32)
    kv_b = np.asarray(kv_b, dtype=np.float32)
    proj_w = np.asarray(proj_w, dtype=np.float32)
    proj_b = np.asarray(proj_b, dtype=np.float32)

    # chunks of 128 src positions that are fully masked in EVERY batch can be
    # dropped at compile time; everything else is handled by the additive mask
    mchunk = mask.reshape(B, NS // P, P)
    dead = mchunk.all(axis=2).all(axis=0)            # [16]
    kept = [c for c in range(NS // P) if not dead[c]]
    if not kept:
        kept = [0]
    nk = len(kept)

    nc = _NC_CACHE.get(nk)
    if nc is None:
        nc = _build_nc(nk)
        _NC_CACHE[nk] = nc

    maskadd = np.where(mask, np.float32(NEG), np.float32(0.0)).astype(np.float32)
    bf = ml_dtypes.bfloat16

    in_maps = []
    for c in range(2 * B):
        b, g = c // 2, c % 2
        gs, ge = g * CH, (g + 1) * CH
        keep_pos = np.concatenate([np.arange(c * P, (c + 1) * P) for c in kept])
        in_maps.append({
            "tgtT": np.ascontiguousarray(tgt[b].T).astype(bf),
            "srcT": np.ascontiguousarray(src[b].T[:, keep_pos]).astype(bf),
            "qwT": np.ascontiguousarray(q_w[gs:ge].T).astype(bf),
            "kwT": np.ascontiguousarray(kv_w[gs:ge].T).astype(bf),
            "vwT": np.ascontiguousarray(kv_w[D + gs:D + ge].T).astype(bf),
            "pwT": np.ascontiguousarray(proj_w[:, gs:ge].T).astype(bf),
            "qb": q_b[gs:ge].copy(),
            "kb": kv_b[gs:ge].copy(),
            "vb": kv_b[D + gs:D + ge].copy(),
            "maskT": np.ascontiguousarray(maskadd[b][keep_pos].reshape(nk, P).T),
        })

    res = run_bass_kernel_spmd(nc, in_maps, list(range(2 * B)),
                               **(_run_kwargs or {}))
    if _run_kwargs:
        kernel.last_result = res

    out = np.empty((B, NT, D), dtype=np.float32)
    for b in range(B):
        part = (res.results[2 * b]["outT"].astype(np.float32)
                + res.results[2 * b + 1]["outT"].astype(np.float32))
        out[b] = part.T + proj_b
    return out


# revision 42
# speedup vs baseline: 1.0159x; 1.0159x over previous
"""Trainium2 Bass kernel for nn_CrossAttention_84310208020733.

Cross-attention: out = proj(softmax(mask(q @ k^T * scale)) @ v), with
  q = tgt @ q_w.T + q_b               [B=4, NT=1024, D=1024]
  k, v = split(src @ kv_w.T + kv_b)   [B=4, NS=2048, D=1024], H=16 heads, Dh=64

Sharding over 8 NeuronCores: core c handles batch b = c//2 and head group
g = c%2 (8 heads = 512 channels).  Each core computes its partial
proj-output (contraction over its 512 attn channels) in transposed layout
[out_ch, rows]; the host sums the two partials per batch, transposes, and
adds proj_b (the "all-reduce after proj" done at gather time).

On-device layout is feature-major throughout ("T" = channels on SBUF
partitions):
  qT = qwT.T @ tgtT       [512, 1024]
  kT = kwT.T @ srcT       [512, NS_kept]
  v  = srcT.T @ vwT       [NS_kept, 512]   (+ ones/zero columns for row-sums)
  sT = kT_h.T @ qT_h      [src 128, rows 512] per head pair (row-packed K=64)
  pT = exp(sT * scale + maskbias)  (ACT, bf16 out; no max-subtraction)
  av = [v_h | 1].T @ pT   -> [Dh(+1), rows] unnormalized out + row sums
  oT = av * bcast(1/sum)  [512, 1024]
  outT = pwT.T @ oT       [1024, 1024] partial, bf16

Perf notes (measured on HW):
  - serial K=128 matmul chains run at ~318 ns per 512-col stream; two
    interleaved independent chains run at ~263 ns; K=64 quad pairs
    (tile_position (0,0)/(64,0)) co-stream at ~127 ns per matmul.  All
    projection chains are therefore emitted as interleaved pairs.
  - DMAs are issued in critical-path order (small consts, then per-k
    w_q/tgt-n0 pairs, w_k/src-col0 pairs, ...) so the first attention
    block starts ~15 us in instead of ~47 us.
  - GPSIMD cannot touch PSUM; fp8 fails the 2e-2 accuracy gate; both
    were measured/verified and rejected.

Fully-masked 128-wide src chunks (per the runtime mask, intersected across
batches) are dropped at compile time; partial masks are handled via the
additive -30000 bias inside the exp activation.
"""

import numpy as np
import ml_dtypes

import concourse.bass as bass
import concourse.bacc as bacc
import concourse.tile as tile
from concourse import mybir
from concourse.bass_utils import run_bass_kernel_spmd

P = 128
B = 4
NT = 1024
NS = 2048
D = 1024
H = 16
DH = 64
G = 2              # head groups (tensor-parallel dim)
HG = H // G        # heads per core = 8
CH = HG * DH       # channels per core = 512
KO = D // P        # 8 contraction chunks for the projections
CHO = CH // P      # 4 channel tiles per core
SCALE = DH ** -0.5
NEG = -30000.0
BF16 = mybir.dt.bfloat16
F32 = mybir.dt.float32
EXP = mybir.ActivationFunctionType.Exp

# vaug per-pair block: [A: 64 ch + 1 ones][B: 1 ones + 63 zero + 64 ch]
ABLK = DH + 1            # 65
BBLK = P                 # 128
PBLK = ABLK + BBLK       # 193


def _build_nc(nk: int) -> "bacc.Bacc":
    """Emit the per-core program for nk kept 128-wide source chunks."""
    ns_k = nk * P
    NSB = ns_k // 512 if ns_k % 512 == 0 else ns_k // 512 + 1
    nc = bacc.Bacc("TRN2", target_bir_lowering=False, debug=False)

    tgtT = nc.dram_tensor("tgtT", [D, NT], BF16, kind="ExternalInput")
    srcT = nc.dram_tensor("srcT", [D, ns_k], BF16, kind="ExternalInput")
    qwT = nc.dram_tensor("qwT", [D, CH], BF16, kind="ExternalInput")
    kwT = nc.dram_tensor("kwT", [D, CH], BF16, kind="ExternalInput")
    vwT = nc.dram_tensor("vwT", [D, CH], BF16, kind="ExternalInput")
    pwT = nc.dram_tensor("pwT", [CH, D], BF16, kind="ExternalInput")
    qb = nc.dram_tensor("qb", [CH], F32, kind="ExternalInput")
    kb = nc.dram_tensor("kb", [CH], F32, kind="ExternalInput")
    vb = nc.dram_tensor("vb", [CH], F32, kind="ExternalInput")
    maskT = nc.dram_tensor("maskT", [P, nk], F32, kind="ExternalInput")
    outT = nc.dram_tensor("outT", [D, NT], BF16, kind="ExternalOutput")

    with tile.TileContext(nc) as tc:
        with (
            tc.tile_pool(name="persist", bufs=1) as pers,
            tc.tile_pool(name="work", bufs=3) as work,
            tc.tile_pool(name="ps", bufs=2, space="PSUM") as ps,
        ):
            # ---- persistent tiles ------------------------------------
            # DMA issuing is the input-bandwidth ceiling (~616 ns per
            # dma_start, ~12.8 GB/s per DMA engine): keep per-k splits so
            # transfers spread over many engines, and round-robin the issue
            # cost over four otherwise-idle sequencers in priority order.
            mask_t = pers.tile([P, nk], F32, tag="mask_t")
            nc.scalar.dma_start(out=mask_t[:], in_=maskT.ap())
            qb_t = pers.tile([P, CHO], F32, tag="qb_t")
            nc.scalar.dma_start(out=qb_t[:], in_=qb.ap().rearrange("(o p) -> p o", p=P))
            kb_t = pers.tile([P, CHO], F32, tag="kb_t")
            nc.scalar.dma_start(out=kb_t[:], in_=kb.ap().rearrange("(o p) -> p o", p=P))
            vb_bc = pers.tile([P, CH], F32, tag="vb_bc")
            vb_ap = vb.ap()
            vb_bcast_src = bass.AP(tensor=vb_ap.tensor, offset=vb_ap.offset,
                                   ap=[[0, P]] + list(vb_ap.ap))
            nc.scalar.dma_start(out=vb_bc[:], in_=vb_bcast_src)

            w_q = pers.tile([P, KO, CH], BF16, tag="w_q")
            tgt_t = pers.tile([P, KO, NT], BF16, tag="tgt_t")
            w_k = pers.tile([P, KO, CH], BF16, tag="w_k")
            src_t = pers.tile([P, KO, ns_k], BF16, tag="src_t")
            w_v = pers.tile([P, KO, CH], BF16, tag="w_v")
            w_p = pers.tile([P, CHO, D], BF16, tag="w_p")
            qT = pers.tile([P, CHO, NT], BF16, tag="qT")
            kT = pers.tile([P, CHO, ns_k], BF16, tag="kT")
            oT = pers.tile([P, CHO, NT], BF16, tag="oT")

            # Each issuing engine's queues process its own backlog roughly
            # concurrently, so priority only holds WITHIN one engine's issue
            # order.  Assign whole waves per engine: sync owns the q-side
            # critical path, gpsimd owns the k/v-side src stream.  (Only
            # SP/gpsimd/ACT may issue DMAs; ACT is kept free for the exps.)
            # Each issuing engine feeds ~8 DMA queues (~100 GB/s), and an
            # engine's queued DMAs transfer roughly in issue order.  So: split
            # every wave half/half between sync and gpsimd (both run at full
            # tilt, waves complete in priority order at ~200 GB/s aggregate).
            def dma_wave(out_slices):
                for idx, (out, in_) in enumerate(out_slices):
                    eng = nc.sync if idx % 2 == 0 else nc.gpsimd
                    eng.dma_start(out=out, in_=in_)

            # k=0 slices split 4-ways: a single 128 KB DMA sits on one ~13 GB/s
            # queue for ~10 us, which would gate the very first matmuls
            c0w = min(512, ns_k)
            dma_wave([(w_q[:, 0, c * 128:(c + 1) * 128],
                       qwT.ap()[0:P, c * 128:(c + 1) * 128]) for c in range(4)]
                     + [(tgt_t[:, 0, c * 128:(c + 1) * 128],
                         tgtT.ap()[0:P, c * 128:(c + 1) * 128]) for c in range(4)])
            dma_wave([(w_q[:, k, :], qwT.ap()[k * P:(k + 1) * P, :])
                      for k in range(1, KO)]
                     + [(tgt_t[:, k, 0:512], tgtT.ap()[k * P:(k + 1) * P, 0:512])
                        for k in range(1, KO)])
            dma_wave([(w_k[:, 0, c * 128:(c + 1) * 128],
                       kwT.ap()[0:P, c * 128:(c + 1) * 128]) for c in range(4)]
                     + [(src_t[:, 0, c * 128:(c + 1) * 128],
                         srcT.ap()[0:P, c * 128:(c + 1) * 128])
                        for c in range(min(4, (c0w + 127) // 128))])
            dma_wave([(w_k[:, k, :], kwT.ap()[k * P:(k + 1) * P, :])
                      for k in range(1, KO)]
                     + [(src_t[:, k, 0:c0w],
                         srcT.ap()[k * P:(k + 1) * P, 0:c0w])
                        for k in range(1, KO)])
            dma_wave([(w_v[:, k, :], vwT.ap()[k * P:(k + 1) * P, :])
                      for k in range(KO)])
            for c0 in range(512, ns_k, 512):
                c1 = min(c0 + 512, ns_k)
                dma_wave([(src_t[:, k, c0:c1],
                           srcT.ap()[k * P:(k + 1) * P, c0:c1])
                          for k in range(KO)])
            dma_wave([(tgt_t[:, k, 512:1024],
                       tgtT.ap()[k * P:(k + 1) * P, 512:1024])
                      for k in range(KO)])
            dma_wave([(w_p[:, o, :], pwT.ap()[o * P:(o + 1) * P, :])
                      for o in range(CHO)])

            vaug = [pers.tile([P, HG // 2 * PBLK], BF16, tag=f"vaug{i}",
                              name=f"vaug{i}")
                    for i in range(nk)]

            # ---- emission units --------------------------------------
            # Interleaved pair-chains: emit two independent K-accumulation
            # chains with their matmuls alternated so the PE pipelines the
            # weight loads (measured ~263 ns vs ~318 ns per matmul).
            def _mm_pair(specs):
                # specs: list of (pmm, lhsT_fn, rhs_fn, n_k) per chain
                depth = max(s[3] for s in specs)
                for k in range(depth):
                    for pmm, lf, rf, nkk in specs:
                        if k < nkk:
                            nc.tensor.matmul(pmm[:], lf(k), rf(k),
                                             start=(k == 0), stop=(k == nkk - 1))

            def qt_chain(m, n):
                pmm = ps.tile([P, 512], F32, tag="acc", bufs=2, name="pmm_q")
                return (pmm,
                        lambda k: w_q[:, k, m * P:(m + 1) * P],
                        lambda k: tgt_t[:, k, n * 512:(n + 1) * 512], KO,
                        lambda: nc.vector.tensor_scalar_add(
                            qT[:, m, n * 512:(n + 1) * 512], pmm[:],
                            qb_t[:, m:m + 1]))

            def kt_chain(m, n):
                c0, c1 = n * 512, min((n + 1) * 512, ns_k)
                pmm = ps.tile([P, 512], F32, tag="acc", bufs=2, name="pmm_k")
                pv = pmm[:, 0:c1 - c0]
                return (pv,
                        lambda k: w_k[:, k, m * P:(m + 1) * P],
                        lambda k: src_t[:, k, c0:c1], KO,
                        lambda: nc.vector.tensor_scalar_add(
                            kT[:, m, c0:c1], pv[:], kb_t[:, m:m + 1]))

            def v_chain(ms):
                pmm = ps.tile([P, 512], F32, tag="acc", bufs=2, name="pmm_v")

                def fin():
                    va = vaug[ms].rearrange("p (t c) -> p t c", c=PBLK)
                    pv = pmm.rearrange("p (t c) -> p t c", c=2 * DH)
                    vv = vb_bc.rearrange("p (t c) -> p t c", c=2 * DH)
                    nc.vector.tensor_add(va[:, :, 0:DH], pv[:, :, 0:DH],
                                         vv[:, :, 0:DH])
                    nc.vector.tensor_add(va[:, :, ABLK + DH:PBLK],
                                         pv[:, :, DH:2 * DH],
                                         vv[:, :, DH:2 * DH])
                    nc.vector.memset(va[:, :, DH:DH + 1], 1.0)
                    nc.vector.memset(va[:, :, ABLK:ABLK + 1], 1.0)
                    nc.vector.memset(va[:, :, ABLK + 1:ABLK + DH], 0.0)
                return (pmm,
                        lambda k: src_t[:, k, ms * P:(ms + 1) * P],
                        lambda k: w_v[:, k, :], KO, fin)

            def proj_chain(m, n, tag="acc", act_copy=False, pmm=None):
                if pmm is None:
                    pmm = ps.tile([P, 512], F32, tag=tag, bufs=2, name="pmm_p")

                def fin():
                    ob = work.tile([P, 512], BF16, tag="ob", bufs=4)
                    if act_copy:
                        nc.scalar.copy(ob[:], pmm[:])
                    else:
                        nc.vector.tensor_copy(ob[:], pmm[:])
                    # 4 column sub-DMAs: one 128 KB DMA would sit on a single
                    # ~13 GB/s queue for ~10 us.  n=0 stays on sync (gpsimd
                    # runs the norm broadcasts mid-sweep); the tail rotates
                    # over all three issuers.
                    for c in range(4):
                        eng = ([nc.sync, nc.gpsimd, nc.scalar][(m * 4 + c) % 3]
                               if n == 1 else nc.sync)
                        eng.dma_start(
                            out=outT.ap()[m * P:(m + 1) * P,
                                          n * 512 + c * 128:n * 512 + (c + 1) * 128],
                            in_=ob[:, c * 128:(c + 1) * 128])
                return (pmm,
                        lambda k: w_p[:, k, m * P:(m + 1) * P],
                        lambda k: oT[:, k, n * 512:(n + 1) * 512], CHO, fin)

            def pair_unit(*chains):
                def run():
                    ss = [c() for c in chains if c is not None]
                    _mm_pair([s[0:4] for s in ss])
                    for s in ss:
                        s[4]()
                    return True
                return run

            # filler queues: PE work woven between the attention chunks.
            crit = []
            lazy = []

            def drain(nu):
                # pop until a unit emits real work (qt1 units may be no-ops)
                for _ in range(nu):
                    while True:
                        q = crit if crit else lazy
                        if not q:
                            return
                        if q.pop(0)():
                            break

            def attn_block(t, n, v_weave=False, drain_js=(2, 6, 10),
                           pre_norm=None):
                # Emission in chunk pairs: both chunks' score quads go out
                # back-to-back (K=64 quads co-stream across adjacent matmuls),
                # then the av matmuls of the pair from two chunks ago (the lag
                # also absorbs late vaug/w_v arrival in block 0).
                rsl = slice(n * 512, (n + 1) * 512)
                avA = ps.tile([ABLK, 512], F32, tag="av", bufs=2, name="avA")
                avB = ps.tile([P, 512], F32, tag="av", bufs=2, name="avB")
                pts = {}

                def scores_exp(j):
                    st = ps.tile([P, 1024], F32, tag="st", name="st")
                    nc.tensor.matmul(
                        st[:, 0:512], kT[0:DH, t, j * P:(j + 1) * P],
                        qT[0:DH, t, rsl], start=True, stop=True,
                        tile_position=(0, 0))
                    nc.tensor.matmul(
                        st[:, 512:1024], kT[DH:P, t, j * P:(j + 1) * P],
                        qT[DH:P, t, rsl], start=True, stop=True,
                        tile_position=(64, 0))
                    pt = work.tile([P, 1024], BF16, tag="pt", bufs=6, name="pt")
                    nc.scalar.activation(out=pt[:], in_=st[:], func=EXP,
                                         bias=mask_t[:, j:j + 1], scale=SCALE)
                    pts[j] = pt

                def av(j):
                    pt = pts.pop(j)
                    va = vaug[j].rearrange("p (t c) -> p t c", c=PBLK)
                    nc.tensor.matmul(avA[:], va[:, t, 0:ABLK], pt[:, 0:512],
                                     start=(j == 0), stop=(j == nk - 1))
                    nc.tensor.matmul(avB[:], va[:, t, ABLK:PBLK],
                                     pt[:, 512:1024],
                                     start=(j == 0), stop=(j == nk - 1))

                # av lags 2 chunks (absorbs late vaug in block 0 and the
                # previous block's norm holding the av PSUM ring); the filler
                # drain sits before the avs so the PE has work if av waits
                for j0 in range(0, nk, 2):
                    j1 = min(j0 + 1, nk - 1)
                    scores_exp(j0)
                    if j1 > j0:
                        scores_exp(j1)
                    if v_weave:
                        pair_unit(mk(v_chain, j0),
                                  mk(v_chain, j1) if j1 > j0 else None)()
                    if j0 in drain_js:
                        drain(1)
                    for j in (j0 - 2, j0 - 1):
                        if j >= 0:
                            av(j)
                for j in sorted(pts):
                    av(j)
                if pre_norm is not None:
                    pre_norm()
                # normalization — no PE involvement: reciprocal of the [1,512]
                # sums row, Pool partition-broadcast, then a PSUM-direct mult.
                # Keeping the PE queue free here removes the inter-block bubble
                # where the next block's scores sat behind broadcast matmuls.
                halves = ((avA, avA[DH:DH + 1, :], avA[0:DH, :], 0, DH),
                          (avB, avB[0:1, :], avB[DH:P, :], DH, P))
                sums, rbrow, rb = [], [], []
                for _, srow, _, _, _ in halves:
                    su = work.tile([1, 512], F32, tag="sums", bufs=4,
                                   name="sums")
                    nc.vector.tensor_copy(su[:], srow)
                    sums.append(su)
                for su in sums:
                    rr = work.tile([1, 512], F32, tag="rbrow", bufs=4,
                                   name="rbrow")
                    nc.vector.reciprocal_approx_fast(rr[:], su[:])
                    rbrow.append(rr)
                for rr in rbrow:
                    rbt = work.tile([P, 512], F32, tag="rb", bufs=4, name="rb")
                    nc.gpsimd.partition_broadcast(rbt[:], rr[:])
                    rb.append(rbt)
                for (acc, _, data, r0, r1), rbt in zip(halves, rb):
                    nc.vector.tensor_mul(oT[r0:r1, t, rsl], data, rbt[r0:r1, :])

            # ---- schedule --------------------------------------------
            def mk(f, *a):
                return lambda: f(*a)

            qt1_done = [False] * CHO

            def qt1_pair(t):
                # covers qt(t,1) and qt(t+1,1) when available
                todo = [tt for tt in (t, t + 1) if tt < CHO and not qt1_done[tt]]
                for tt in todo:
                    qt1_done[tt] = True
                if todo:
                    pair_unit(*[mk(qt_chain, tt, 1) for tt in todo])()
                return bool(todo)

            # head: the window is DMA-paced.  qt(0,0) first (its inputs are
            # wave 1), then the other qt(t,0) chains fill the PE while wave 2
            # (kt inputs) streams in; kt(0,0) follows; kt(0,1)/kt(0,2) go to
            # the crit queue, drained inside block 0 before chunk 4 needs them.
            pair_unit(mk(qt_chain, 0, 0))()
            for t2 in range(1, CHO, 2):
                pair_unit(mk(qt_chain, t2, 0),
                          mk(qt_chain, t2 + 1, 0) if t2 + 1 < CHO else None)()
            pair_unit(mk(kt_chain, 0, 0))()
            for x in range(1, NSB, 2):
                crit.append(pair_unit(
                    mk(kt_chain, 0, x),
                    mk(kt_chain, 0, x + 1) if x + 1 < NSB else None))

            # n=0 sweep
            for t in range(CHO):
                if t + 1 < CHO:
                    for x in range(0, NSB, 2):
                        crit.append(pair_unit(
                            mk(kt_chain, t + 1, x),
                            mk(kt_chain, t + 1, x + 1) if x + 1 < NSB else None))
                lazy.append(mk(qt1_pair, t))
                attn_block(t, 0, v_weave=(t == 0))
                while crit:
                    crit.pop(0)()

            # n=1 sweep with n=0 projection woven in (one real unit per
            # ACT-bound block; the smart drain skips spent qt1 entries)
            for m in range(0, KO, 2):
                lazy.append(pair_unit(mk(proj_chain, m, 0),
                                      mk(proj_chain, m + 1, 0)))
            for t in range(CHO):
                qt1_pair(t)
                if t == CHO - 1:
                    # overlap the last block's normalization with four tail
                    # chains' partial contraction (k < CHO-1 only touches oT
                    # written by earlier blocks; m2/m3 borrow dead st slots)
                    head_specs = []

                    def tail_head():
                        # six chains' k<CHO-1 partials run while the last
                        # norm's DVE/Pool chain executes; m2-m5 pack two per
                        # dead st PSUM slot
                        st1 = ps.tile([P, 1024], F32, tag="st", name="tp1")
                        st2 = ps.tile([P, 1024], F32, tag="st", name="tp2")
                        over = [None, None, st1[:, 0:512], st1[:, 512:1024],
                                st2[:, 0:512], st2[:, 512:1024]]
                        for m in range(6):
                            head_specs.append(
                                proj_chain(m, 1, "acc", m % 2 == 1,
                                           pmm=over[m]))
                        for k in range(CHO - 1):
                            for s in head_specs:
                                nc.tensor.matmul(s[0][:], s[1](k), s[2](k),
                                                 start=(k == 0), stop=False)
                    attn_block(t, 1, drain_js=(6,), pre_norm=tail_head)
                else:
                    attn_block(t, 1, drain_js=(6,))
            while lazy:
                lazy.pop(0)()
            for s in head_specs:
                nc.tensor.matmul(s[0][:], s[1](CHO - 1), s[2](CHO - 1),
                                 start=False, stop=True)
            for s in head_specs:
                s[4]()
            pair_unit(mk(proj_chain, 6, 1, "av", True),
                      mk(proj_chain, 7, 1, "av", False))()
    nc.compile()
    return nc


_NC_CACHE: dict[int, "bacc.Bacc"] = {}


def kernel(tgt, src, src_padded_mask, q_w, q_b, kv_w, kv_b, proj_w, proj_b,
           _run_kwargs: dict | None = None):
    tgt = np.asarray(tgt, dtype=np.float32)
    src = np.asarray(src, dtype=np.float32)
    mask = np.asarray(src_padded_mask).astype(bool)
    q_w = np.asarray(q_w, dtype=np.float32)
    q_b = np.asarray(q_b, dtype=np.float32)
    kv_w = np.asarray(kv_w, dtype=np.float32)
    kv_b = np.asarray(kv_b, dtype=np.float32)
    proj_w = np.asarray(proj_w, dtype=np.float32)
    proj_b = np.asarray(proj_b, dtype=np.float32)

    # chunks of 128 src positions that are fully masked in EVERY batch can be
    # dropped at compile time; everything else is handled by the additive mask
    mchunk = mask.reshape(B, NS // P, P)
    dead = mchunk.all(axis=2).all(axis=0)            # [16]
    kept = [c for c in range(NS // P) if not dead[c]]
    if not kept:
        kept = [0]
    nk = len(kept)

    nc = _NC_CACHE.get(nk)
    if nc is None:
        nc = _build_nc(nk)
        _NC_CACHE[nk] = nc

    maskadd = np.where(mask, np.float32(NEG), np.float32(0.0)).astype(np.float32)
    bf = ml_dtypes.bfloat16

    in_maps = []
    for c in range(2 * B):
        b, g = c // 2, c % 2
        gs, ge = g * CH, (g + 1) * CH
        keep_pos = np.concatenate([np.arange(c * P, (c + 1) * P) for c in kept])
        in_maps.append({
            "tgtT": np.ascontiguousarray(tgt[b].T).astype(bf),
            "srcT": np.ascontiguousarray(src[b].T[:, keep_pos]).astype(bf),
            "qwT": np.ascontiguousarray(q_w[gs:ge].T).astype(bf),
            "kwT": np.ascontiguousarray(kv_w[gs:ge].T).astype(bf),
            "vwT": np.ascontiguousarray(kv_w[D + gs:D + ge].T).astype(bf),
            "pwT": np.ascontiguousarray(proj_w[:, gs:ge].T).astype(bf),
            "qb": q_b[gs:ge].copy(),
            "kb": kv_b[gs:ge].copy(),
            "vb": kv_b[D + gs:D + ge].copy(),
            "maskT": np.ascontiguousarray(maskadd[b][keep_pos].reshape(nk, P).T),
        })

    res = run_bass_kernel_spmd(nc, in_maps, list(range(2 * B)),
                               **(_run_kwargs or {}))
    if _run_kwargs:
        kernel.last_result = res

    out = np.empty((B, NT, D), dtype=np.float32)
    for b in range(B):
        part = (res.results[2 * b]["outT"].astype(np.float32)
                + res.results[2 * b + 1]["outT"].astype(np.float32))
        out[b] = part.T + proj_b
    return out
